# revision 16
# baseline (speedup 1.0000x reference)
"""ACE loss kernel for 8 Trainium2 NeuronCores (data-parallel over batch rows).

Host: tiny SVD whitening setup (D x D, replicating the reference's jax-f32
SVD exactly), signature whitening, sharding/layout. Device (per core, 8192
rows): Y^T = W^T X^T - mu_w (fp16 matmuls, f32 PSUM), row norms via Gram
diagonal + Newton rsqrt, ACE = (Y/||Y||) @ sHat^T, fused exp+rowsum for
logsumexp, per-row label gather via tensor_mask_reduce. Host: assemble ACE,
loss = -mean(num - ln(sumexp)).
"""
import sys, types
import numpy as np

# ---------------------------------------------------------------------------
# environment: make concourse importable + register the NTFF profile hook
# ---------------------------------------------------------------------------
if "/opt/trn_rl_repo" not in sys.path:
    sys.path.insert(0, "/opt/trn_rl_repo")

if "antenv.axon_hooks" not in sys.modules:
    _hooks_mod = types.ModuleType("antenv.axon_hooks")
    _hook_holder = {"hook": None}
    _hooks_mod.set_axon_ntff_profile_hook = lambda h: _hook_holder.__setitem__("hook", h)
    _hooks_mod.get_axon_ntff_profile_hook = lambda: _hook_holder["hook"]
    sys.modules["antenv.axon_hooks"] = _hooks_mod
    try:
        from trn_agent_boot.trn_boot import _ntff_profile_via_ctypes
        _hooks_mod.set_axon_ntff_profile_hook(
            _ntff_profile_via_ctypes("/opt/axon/libaxon_pjrt.so"))
    except Exception:
        pass

from contextlib import ExitStack

import concourse.bass as bass
import concourse.tile as tile
from concourse import bacc, mybir
from concourse.bass_utils import run_bass_kernel_spmd

B, C, D = 65536, 1000, 256
N_CORES = 8
B_SHARD = B // N_CORES          # 8192
SUPER = 512                     # rows per supertile
SUB = 128                       # rows per subtile
F32 = mybir.dt.float32
F16 = mybir.dt.float16
I32 = mybir.dt.int32
AF = mybir.ActivationFunctionType
ALU = mybir.AluOpType
QK = 0x5F3759DF                 # rsqrt seed magic
NEG_BIG = -3.0e38


def build(n_rows=B_SHARD, windows=None, evict_act_of8=3):
    """windows: per-subtile [lo, hi) class ranges covering every label in that
    subtile (rows are label-sorted by host_prep); None = full range.
    evict_act_of8: of every 8 subtiles, this many get their PSUM->SBUF ACE
    eviction on ScalarE, the rest on VectorE."""
    assert n_rows % SUPER == 0
    n_super = n_rows // SUPER
    n_sub = n_rows // SUB
    if windows is None:
        windows = [(0, C)] * n_sub
    nc = bacc.Bacc("TRN2", target_bir_lowering=False, debug=False,
                   num_devices=N_CORES)

    xt_d = nc.dram_tensor("xt", [2, 128, n_rows], F16, kind="ExternalInput")
    wsb_d = nc.dram_tensor("wsb", [128, 512], F16, kind="ExternalInput")
    sht_d = nc.dram_tensor("sht", [128, 2 * C], F16, kind="ExternalInput")
    muw_d = nc.dram_tensor("muw", [128, 2], F32, kind="ExternalInput")  # -mu_w
    lab_d = nc.dram_tensor("lab", [128, n_sub], F32, kind="ExternalInput")
    iota_d = nc.dram_tensor("iota", [128, C], F16, kind="ExternalInput")
    dmask_d = nc.dram_tensor("dmask", [128, 512], F32, kind="ExternalInput")
    ace_d = nc.dram_tensor("ace", [n_rows, C], F32, kind="ExternalOutput")
    se_d = nc.dram_tensor("sumexp", [128, n_sub], F32, kind="ExternalOutput")
    ne_d = nc.dram_tensor("numv", [128, n_sub], F32, kind="ExternalOutput")

    with tile.TileContext(nc) as tc, ExitStack() as ctx:
        cpool = ctx.enter_context(tc.tile_pool(name="const", bufs=1))
        xpool = ctx.enter_context(tc.tile_pool(name="xt", bufs=4))
        ypool = ctx.enter_context(tc.tile_pool(name="yb", bufs=4))
        apool = ctx.enter_context(tc.tile_pool(name="acesb", bufs=4))
        mpool = ctx.enter_context(tc.tile_pool(name="mask", bufs=3))
        spool = ctx.enter_context(tc.tile_pool(name="small", bufs=4))
        pre_pool = ctx.enter_context(tc.tile_pool(name="prepsum", bufs=1, space="PSUM"))
        aps_pool = ctx.enter_context(tc.tile_pool(name="aceps", bufs=3, space="PSUM"))

        # constants
        wsb = cpool.tile([128, 512], F16, tag="wsb")
        nc.sync.dma_start(wsb[:, :], wsb_d.ap()[:, :])
        sht = cpool.tile([128, 2 * C], F16, tag="sht")
        nc.sync.dma_start(sht[:, :], sht_d.ap()[:, :])
        muw = cpool.tile([128, 2], F32, tag="muw")
        nc.sync.dma_start(muw[:, :], muw_d.ap()[:, :])
        lab = cpool.tile([128, n_sub], F32, tag="lab")
        nc.sync.dma_start(lab[:, :], lab_d.ap()[:, :])
        iota = cpool.tile([128, C], F16, tag="iota")
        nc.sync.dma_start(iota[:, :], iota_d.ap()[:, :])
        dmask = cpool.tile([128, 512], F32, tag="dmask")
        nc.sync.dma_start(dmask[:, :], dmask_d.ap()[:, :])
        se_acc = cpool.tile([128, n_sub], F32, tag="seacc")
        ne_raw = cpool.tile([128, n_sub], F32, tag="neraw")
        ne_acc = cpool.tile([128, n_sub], F32, tag="neacc")
        inv_all = cpool.tile([128, n_sub], F32, tag="invall")
        # junk sinks (written, never read)
        junkd = cpool.tile([128, 128], F16, tag="junkd")
        junkn = cpool.tile([128, C], F16, tag="junkn")
        junke = cpool.tile([128, C], F16, tag="junke")

        def prelude(t):
            # load X^T tiles (feature-major fp16), 2 k-chunks
            xts = []
            for k in range(2):
                xt = xpool.tile([128, SUPER], F16, tag=f"xt{k}")
                nc.sync.dma_start(xt[:, :], xt_d.ap()[k, :, t * SUPER:(t + 1) * SUPER])
                xts.append(xt)

            # mm1: Y^T (pre-bias) [2x128 feats, SUPER rows]
            yps = pre_pool.tile([128, 1024], F32, tag="pre")
            for m in range(2):
                o = yps[:, m * 512:(m + 1) * 512]
                for k in range(2):
                    nc.tensor.matmul(
                        o, lhsT=wsb[:, k * 256 + m * 128: k * 256 + m * 128 + 128],
                        rhs=xts[k][:, :], start=(k == 0), stop=(k == 1))

            # evict Y^T to SBUF fp16, folding in the -mu_w bias
            yb = ypool.tile([128, 1024], F16, tag="yb")
            for m in range(2):
                nc.vector.tensor_scalar(
                    out=yb[:, m * 512:(m + 1) * 512],
                    in0=yps[:, m * 512:(m + 1) * 512],
                    scalar1=muw[:, m:m + 1], scalar2=None, op0=ALU.add)

            # row norms^2 via Gram diagonals
            gram_full = pre_pool.tile([128, 1024], F32, tag="pre")
            gram = gram_full[:, 0:512]
            for u in range(4):
                for k in range(2):
                    sl = yb[:, k * 512 + u * 128: k * 512 + u * 128 + 128]
                    nc.tensor.matmul(gram[:, u * 128:(u + 1) * 128],
                                     lhsT=sl, rhs=sl,
                                     start=(k == 0), stop=(k == 1))
            n2 = spool.tile([128, 4], F32, tag="n2")
            for u in range(4):
                nc.vector.scalar_tensor_tensor(
                    out=junkd[:, :], in0=gram[:, u * 128:(u + 1) * 128],
                    scalar=1.0, in1=dmask[:, u * 128:(u + 1) * 128],
                    op0=ALU.bypass, op1=ALU.mult, accum_out=n2[:, u:u + 1])

            # inv_norm = rsqrt(n2): quake seed + 1 Newton iteration
            t1 = spool.tile([128, 4], I32, tag="nt1")
            nc.vector.tensor_scalar(out=t1[:, :], in0=n2[:, :].bitcast(I32),
                                    scalar1=1, scalar2=None,
                                    op0=ALU.arith_shift_right)
            t2 = spool.tile([128, 4], I32, tag="nt2")
            nc.vector.tensor_scalar(out=t2[:, :], in0=t1[:, :],
                                    scalar1=QK, scalar2=None,
                                    op0=ALU.subtract)          # t - K
            t3 = spool.tile([128, 4], I32, tag="nt3")
            nc.vector.tensor_scalar(out=t3[:, :], in0=t2[:, :],
                                    scalar1=-1, scalar2=None,
                                    op0=ALU.bitwise_xor)              # ~(t-K)
            y = spool.tile([128, 4], F32, tag="ny0")
            nc.vector.tensor_scalar(out=y[:, :].bitcast(I32), in0=t3[:, :],
                                    scalar1=1, scalar2=None,
                                    op0=ALU.add)                       # K - t
            a = spool.tile([128, 4], F32, tag="na")
            nc.vector.tensor_mul(a[:, :], y[:, :], y[:, :])
            b = spool.tile([128, 4], F32, tag="nb")
            nc.vector.tensor_mul(b[:, :], n2[:, :], a[:, :])
            c = spool.tile([128, 4], F32, tag="nc")
            nc.vector.tensor_scalar(out=c[:, :], in0=b[:, :],
                                    scalar1=-0.5, scalar2=1.5,
                                    op0=ALU.mult, op1=ALU.add)
            inv = inv_all[:, t * 4:(t + 1) * 4]
            nc.vector.tensor_mul(inv, y[:, :], c[:, :])
            return yb

        def epilogue(t, yb):
            for u in range(4):
                s = t * 4 + u
                aps = aps_pool.tile([128, 1024], F32, tag="aps")
                for k in range(2):
                    sl = yb[:, k * 512 + u * 128: k * 512 + u * 128 + 128]
                    nc.tensor.matmul(aps[:, 0:512], lhsT=sl,
                                     rhs=sht[:, k * C: k * C + 512],
                                     start=(k == 0), stop=(k == 1))
                    nc.tensor.matmul(aps[:, 512:1000], lhsT=sl,
                                     rhs=sht[:, k * C + 512: k * C + C],
                                     start=(k == 0), stop=(k == 1))

                lo, hi = windows[s]
                w = hi - lo
                ace = apool.tile([128, C], F32, tag="ace")
                iv = inv_all[:, s:s + 1]
                if (s % 8) < evict_act_of8:
                    nc.scalar.activation(ace[:, :], aps[:, 0:C], AF.Copy,
                                         scale=iv)
                else:
                    nc.vector.tensor_scalar(out=ace[:, :], in0=aps[:, 0:C],
                                            scalar1=iv,
                                            scalar2=None, op0=ALU.mult)
                nc.scalar.activation(junke[:, :], aps[:, 0:C], AF.Exp,
                                     scale=iv,
                                     accum_out=se_acc[:, s:s + 1])
                mask = mpool.tile([128, C], F16, tag="mask")
                nc.vector.tensor_scalar(out=mask[:, 0:w], in0=iota[:, lo:hi],
                                        scalar1=lab[:, s:s + 1], scalar2=None,
                                        op0=ALU.is_equal)
                nc.vector.scalar_tensor_tensor(
                    out=junkn[:, 0:w], in0=aps[:, lo:hi], scalar=1.0,
                    in1=mask[:, 0:w],
                    op0=ALU.bypass, op1=ALU.mult, accum_out=ne_raw[:, s:s + 1])

                nc.sync.dma_start(
                    ace_d.ap()[t * SUPER + u * SUB: t * SUPER + (u + 1) * SUB, :],
                    ace[:, :])

        # software pipeline: prelude(t+1) is emitted before epilogue(t) so the
        # norm/inv chain of the next supertile overlaps this one's epilogue
        ybs = {}
        ahead = 3
        for t0 in range(min(ahead, n_super)):
            ybs[t0] = prelude(t0)
        for t in range(n_super):
            if t + ahead < n_super:
                ybs[t + ahead] = prelude(t + ahead)
            epilogue(t, ybs.pop(t))

        nc.vector.tensor_mul(ne_acc[:, :], ne_raw[:, :], inv_all[:, :])
        nc.sync.dma_start(se_d.ap()[:, :], se_acc[:, :])
        nc.sync.dma_start(ne_d.ap()[:, :], ne_acc[:, :])

    nc.compile()
    return nc


def _host_whiten(signatures, b_means, b_covs):
    try:
        import jax
        import jax.numpy as jnp
        with jax.default_device(jax.devices("cpu")[0]):
            cov = jnp.asarray(b_covs) @ jnp.asarray(b_covs).T
            U, eig, _ = jnp.linalg.svd(cov)
            DU = (eig ** -0.5)[:, None] * U.T
            W = DU.T
            mu_w = jnp.asarray(b_means) @ W
            s_w = jnp.asarray(signatures) @ W
            n = jnp.linalg.norm(s_w, axis=1, keepdims=True)
            sHat = s_w / jnp.maximum(n, 1e-12)
            return (np.ascontiguousarray(np.asarray(W, np.float32)),
                    np.asarray(mu_w, np.float32).reshape(1, D),
                    np.asarray(sHat, np.float32))
    except Exception:
        cov = b_covs @ b_covs.T
        U, eig, _ = np.linalg.svd(cov)
        DU = (eig ** -0.5)[:, None] * U.T
        W = np.ascontiguousarray(DU.T)
        mu_w = (b_means @ W).reshape(1, D)
        s_w = signatures @ W
        sHat = s_w / np.maximum(np.linalg.norm(s_w, axis=1, keepdims=True), 1e-12)
        return W, mu_w, sHat


def host_prep(X, labels, signatures, b_means, b_covs, n_rows=B_SHARD):
    """Returns in_maps for run_bass_kernel_spmd."""
    X = np.asarray(X, dtype=np.float32)
    labels = np.asarray(labels).astype(np.int32)
    signatures = np.asarray(signatures, dtype=np.float32)
    b_means = np.asarray(b_means, dtype=np.float32)
    b_covs = np.asarray(b_covs, dtype=np.float32)

    W, mu_w, sHat = _host_whiten(signatures, b_means, b_covs)
    SHT = np.ascontiguousarray(sHat.T)         # [D, C]

    wsb = np.ascontiguousarray(
        W.reshape(2, 128, 256).transpose(1, 0, 2).reshape(128, 512)
    ).astype(np.float16)
    sht = np.ascontiguousarray(
        SHT.reshape(2, 128, C).transpose(1, 0, 2).reshape(128, 2 * C)
    ).astype(np.float16)
    muw = np.ascontiguousarray((-mu_w).reshape(2, 128).T.astype(np.float32))
    iota = np.ascontiguousarray(
        np.tile(np.arange(C, dtype=np.float16)[None, :], (128, 1)))
    dmask = np.ascontiguousarray(np.tile(np.eye(128, dtype=np.float32), (1, 4)))

    n_sub = n_rows // SUB
    in_maps, perms = [], []
    sorted_lab = []
    for i in range(N_CORES):
        xs = X[i * n_rows:(i + 1) * n_rows]
        ls = labels[i * n_rows:(i + 1) * n_rows]
        perm = np.argsort(ls, kind="stable")
        perms.append(perm)
        xs = xs[perm]
        ls = ls[perm]
        sorted_lab.append(ls)
        xt = np.ascontiguousarray(xs.T.astype(np.float16)).reshape(2, 128, n_rows)
        lab = np.ascontiguousarray(ls.reshape(n_sub, 128).T.astype(np.float32))
        in_maps.append({"xt": xt, "wsb": wsb, "sht": sht, "muw": muw,
                        "lab": lab, "iota": iota, "dmask": dmask})
    sl = np.stack(sorted_lab)                       # [cores, n_rows]
    lo = sl.reshape(N_CORES, n_sub, SUB).min(axis=(0, 2))
    hi = sl.reshape(N_CORES, n_sub, SUB).max(axis=(0, 2)) + 1
    windows = tuple((int(a), int(b)) for a, b in zip(lo, hi))
    return in_maps, perms, windows


_CACHE = {}


def run_device(X, labels, signatures, b_means, b_covs, n_rows=B_SHARD,
               trace=False):
    in_maps, perms, windows = host_prep(X, labels, signatures, b_means,
                                        b_covs, n_rows)
    key = (n_rows, windows)
    if key not in _CACHE:
        _CACHE.clear()
        _CACHE[key] = build(n_rows, windows=windows)
    nc = _CACHE[key]
    res = run_bass_kernel_spmd(nc, in_maps, core_ids=list(range(N_CORES)),
                               trace=trace)
    return res, perms


def finish(res, perms, n_rows=B_SHARD):
    ace = np.empty((N_CORES * n_rows, C), np.float32)
    num = np.empty(N_CORES * n_rows, np.float64)
    se = np.empty(N_CORES * n_rows, np.float64)
    for i in range(N_CORES):
        perm = perms[i]
        sl = slice(i * n_rows, (i + 1) * n_rows)
        ace[sl.start:sl.stop][perm] = res.results[i]["ace"]
        num[sl][perm] = res.results[i]["numv"].flatten(order="F").astype(np.float64)
        se[sl][perm] = res.results[i]["sumexp"].flatten(order="F").astype(np.float64)
    loss = -np.mean(num - np.log(se))
    return np.float32(loss), ace


def kernel(X, labels, signatures, b_means, b_covs):
    res, perms = run_device(X, labels, signatures, b_means, b_covs)
    loss, ace = finish(res, perms)
    return loss, ace


# revision 17
# speedup vs baseline: 1.0658x; 1.0658x over previous
"""ACE loss kernel for 8 Trainium2 NeuronCores (data-parallel over batch rows).

Host: tiny SVD whitening setup (D x D, replicating the reference's jax-f32
SVD exactly), signature whitening, sharding/layout. Device (per core, 8192
rows): Y^T = W^T X^T - mu_w (fp16 matmuls, f32 PSUM), row norms via Gram
diagonal + Newton rsqrt, ACE = (Y/||Y||) @ sHat^T, fused exp+rowsum for
logsumexp, per-row label gather via tensor_mask_reduce. Host: assemble ACE,
loss = -mean(num - ln(sumexp)).
"""
import sys, types
import numpy as np

# ---------------------------------------------------------------------------
# environment: make concourse importable + register the NTFF profile hook
# ---------------------------------------------------------------------------
if "/opt/trn_rl_repo" not in sys.path:
    sys.path.insert(0, "/opt/trn_rl_repo")

if "antenv.axon_hooks" not in sys.modules:
    _hooks_mod = types.ModuleType("antenv.axon_hooks")
    _hook_holder = {"hook": None}
    _hooks_mod.set_axon_ntff_profile_hook = lambda h: _hook_holder.__setitem__("hook", h)
    _hooks_mod.get_axon_ntff_profile_hook = lambda: _hook_holder["hook"]
    sys.modules["antenv.axon_hooks"] = _hooks_mod
    try:
        from trn_agent_boot.trn_boot import _ntff_profile_via_ctypes
        _hooks_mod.set_axon_ntff_profile_hook(
            _ntff_profile_via_ctypes("/opt/axon/libaxon_pjrt.so"))
    except Exception:
        pass

from contextlib import ExitStack

import concourse.bass as bass
import concourse.tile as tile
from concourse import bacc, mybir
from concourse.bass_utils import run_bass_kernel_spmd

B, C, D = 65536, 1000, 256
N_CORES = 8
B_SHARD = B // N_CORES          # 8192
SUPER = 512                     # rows per supertile
SUB = 128                       # rows per subtile
F32 = mybir.dt.float32
F16 = mybir.dt.float16
I32 = mybir.dt.int32
AF = mybir.ActivationFunctionType
ALU = mybir.AluOpType
QK = 0x5F3759DF                 # rsqrt seed magic
NEG_BIG = -3.0e38


def build(n_rows=B_SHARD, windows=None, evict_act_of8=3):
    """windows: per-subtile [lo, hi) class ranges covering every label in that
    subtile (rows are label-sorted by host_prep); None = full range.
    evict_act_of8: of every 8 subtiles, this many get their PSUM->SBUF ACE
    eviction on ScalarE, the rest on VectorE."""
    assert n_rows % SUPER == 0
    n_super = n_rows // SUPER
    n_sub = n_rows // SUB
    if windows is None:
        windows = [(0, C)] * n_sub
    nc = bacc.Bacc("TRN2", target_bir_lowering=False, debug=False,
                   num_devices=N_CORES)

    xt_d = nc.dram_tensor("xt", [2, 128, n_rows], F16, kind="ExternalInput")
    wsb_d = nc.dram_tensor("wsb", [128, 512], F16, kind="ExternalInput")
    sht_d = nc.dram_tensor("sht", [128, 2 * C], F16, kind="ExternalInput")
    muw_d = nc.dram_tensor("muw", [128, 2], F32, kind="ExternalInput")  # -mu_w
    lab_d = nc.dram_tensor("lab", [128, n_sub], F32, kind="ExternalInput")
    iota_d = nc.dram_tensor("iota", [128, C], F16, kind="ExternalInput")
    dmask_d = nc.dram_tensor("dmask", [128, 512], F32, kind="ExternalInput")
    ace_d = nc.dram_tensor("ace", [n_rows, C], F32, kind="ExternalOutput")
    se_d = nc.dram_tensor("sumexp", [128, n_sub], F32, kind="ExternalOutput")
    ne_d = nc.dram_tensor("numv", [128, n_sub], F32, kind="ExternalOutput")

    with tile.TileContext(nc) as tc, ExitStack() as ctx:
        cpool = ctx.enter_context(tc.tile_pool(name="const", bufs=1))
        xpool = ctx.enter_context(tc.tile_pool(name="xt", bufs=4))
        ypool = ctx.enter_context(tc.tile_pool(name="yb", bufs=3))
        apool = ctx.enter_context(tc.tile_pool(name="acesb", bufs=4))
        mpool = ctx.enter_context(tc.tile_pool(name="mask", bufs=3))
        spool = ctx.enter_context(tc.tile_pool(name="small", bufs=4))
        pre_pool = ctx.enter_context(tc.tile_pool(name="prepsum", bufs=1, space="PSUM"))
        aps_pool = ctx.enter_context(tc.tile_pool(name="aceps", bufs=3, space="PSUM"))

        # constants
        wsb = cpool.tile([128, 512], F16, tag="wsb")
        nc.sync.dma_start(wsb[:, :], wsb_d.ap()[:, :])
        sht = cpool.tile([128, 2 * C], F16, tag="sht")
        nc.sync.dma_start(sht[:, :], sht_d.ap()[:, :])
        muw = cpool.tile([128, 2], F32, tag="muw")
        nc.sync.dma_start(muw[:, :], muw_d.ap()[:, :])
        lab = cpool.tile([128, n_sub], F32, tag="lab")
        nc.sync.dma_start(lab[:, :], lab_d.ap()[:, :])
        iota = cpool.tile([128, C], F16, tag="iota")
        nc.sync.dma_start(iota[:, :], iota_d.ap()[:, :])
        dmask = cpool.tile([128, 512], F32, tag="dmask")
        nc.sync.dma_start(dmask[:, :], dmask_d.ap()[:, :])
        se_acc = cpool.tile([128, n_sub], F32, tag="seacc")
        ne_raw = cpool.tile([128, n_sub], F32, tag="neraw")
        ne_acc = cpool.tile([128, n_sub], F32, tag="neacc")
        inv_all = cpool.tile([128, n_sub], F32, tag="invall")
        # junk sinks (written, never read)
        junkd = cpool.tile([128, 128], F16, tag="junkd")
        junkn = cpool.tile([128, C], F16, tag="junkn")
        junke = cpool.tile([128, C], F16, tag="junke")

        def prelude(t):
            # load X^T tiles (feature-major fp16), 2 k-chunks
            xts = []
            for k in range(2):
                xt = xpool.tile([128, SUPER], F16, tag=f"xt{k}")
                nc.sync.dma_start(xt[:, :], xt_d.ap()[k, :, t * SUPER:(t + 1) * SUPER])
                xts.append(xt)

            # mm1: Y^T (pre-bias) [2x128 feats, SUPER rows]
            yps = pre_pool.tile([128, 1024], F32, tag="pre")
            for m in range(2):
                o = yps[:, m * 512:(m + 1) * 512]
                for k in range(2):
                    nc.tensor.matmul(
                        o, lhsT=wsb[:, k * 256 + m * 128: k * 256 + m * 128 + 128],
                        rhs=xts[k][:, :], start=(k == 0), stop=(k == 1))

            # evict Y^T to SBUF fp16, folding in the -mu_w bias
            yb = ypool.tile([128, 1024], F16, tag="yb")
            for m in range(2):
                nc.vector.tensor_scalar(
                    out=yb[:, m * 512:(m + 1) * 512],
                    in0=yps[:, m * 512:(m + 1) * 512],
                    scalar1=muw[:, m:m + 1], scalar2=None, op0=ALU.add)

            # row norms^2 via Gram diagonals
            gram_full = pre_pool.tile([128, 1024], F32, tag="pre")
            gram = gram_full[:, 0:512]
            for u in range(4):
                for k in range(2):
                    sl = yb[:, k * 512 + u * 128: k * 512 + u * 128 + 128]
                    nc.tensor.matmul(gram[:, u * 128:(u + 1) * 128],
                                     lhsT=sl, rhs=sl,
                                     start=(k == 0), stop=(k == 1))
            n2 = spool.tile([128, 4], F32, tag="n2")
            for u in range(4):
                nc.vector.scalar_tensor_tensor(
                    out=junkd[:, :], in0=gram[:, u * 128:(u + 1) * 128],
                    scalar=1.0, in1=dmask[:, u * 128:(u + 1) * 128],
                    op0=ALU.bypass, op1=ALU.mult, accum_out=n2[:, u:u + 1])

            # inv_norm = rsqrt(n2): quake seed + 1 Newton iteration
            t1 = spool.tile([128, 4], I32, tag="nt1")
            nc.vector.tensor_scalar(out=t1[:, :], in0=n2[:, :].bitcast(I32),
                                    scalar1=1, scalar2=None,
                                    op0=ALU.arith_shift_right)
            t2 = spool.tile([128, 4], I32, tag="nt2")
            nc.vector.tensor_scalar(out=t2[:, :], in0=t1[:, :],
                                    scalar1=QK, scalar2=None,
                                    op0=ALU.subtract)          # t - K
            t3 = spool.tile([128, 4], I32, tag="nt3")
            nc.vector.tensor_scalar(out=t3[:, :], in0=t2[:, :],
                                    scalar1=-1, scalar2=None,
                                    op0=ALU.bitwise_xor)              # ~(t-K)
            y = spool.tile([128, 4], F32, tag="ny0")
            nc.vector.tensor_scalar(out=y[:, :].bitcast(I32), in0=t3[:, :],
                                    scalar1=1, scalar2=None,
                                    op0=ALU.add)                       # K - t
            a = spool.tile([128, 4], F32, tag="na")
            nc.vector.tensor_mul(a[:, :], y[:, :], y[:, :])
            b = spool.tile([128, 4], F32, tag="nb")
            nc.vector.tensor_mul(b[:, :], n2[:, :], a[:, :])
            c = spool.tile([128, 4], F32, tag="nc")
            nc.vector.tensor_scalar(out=c[:, :], in0=b[:, :],
                                    scalar1=-0.5, scalar2=1.5,
                                    op0=ALU.mult, op1=ALU.add)
            inv = inv_all[:, t * 4:(t + 1) * 4]
            nc.vector.tensor_mul(inv, y[:, :], c[:, :])
            return yb

        def epilogue(t, yb):
            for u in range(4):
                s = t * 4 + u
                aps = aps_pool.tile([128, 1024], F32, tag="aps")
                for k in range(2):
                    sl = yb[:, k * 512 + u * 128: k * 512 + u * 128 + 128]
                    nc.tensor.matmul(aps[:, 0:512], lhsT=sl,
                                     rhs=sht[:, k * C: k * C + 512],
                                     start=(k == 0), stop=(k == 1))
                    nc.tensor.matmul(aps[:, 512:1000], lhsT=sl,
                                     rhs=sht[:, k * C + 512: k * C + C],
                                     start=(k == 0), stop=(k == 1))

                lo, hi = windows[s]
                w = hi - lo
                ace = apool.tile([128, C], F32, tag="ace")
                iv = inv_all[:, s:s + 1]
                if (s % 8) < evict_act_of8:
                    nc.scalar.activation(ace[:, :], aps[:, 0:C], AF.Copy,
                                         scale=iv)
                else:
                    nc.vector.tensor_scalar(out=ace[:, :], in0=aps[:, 0:C],
                                            scalar1=iv,
                                            scalar2=None, op0=ALU.mult)
                nc.scalar.activation(junke[:, :], aps[:, 0:C], AF.Exp,
                                     scale=iv,
                                     accum_out=se_acc[:, s:s + 1])
                mask = mpool.tile([128, C], F16, tag="mask")
                nc.vector.tensor_scalar(out=mask[:, 0:w], in0=iota[:, lo:hi],
                                        scalar1=lab[:, s:s + 1], scalar2=None,
                                        op0=ALU.is_equal)
                nc.vector.scalar_tensor_tensor(
                    out=junkn[:, 0:w], in0=aps[:, lo:hi], scalar=1.0,
                    in1=mask[:, 0:w],
                    op0=ALU.bypass, op1=ALU.mult, accum_out=ne_raw[:, s:s + 1])

                nc.sync.dma_start(
                    ace_d.ap()[t * SUPER + u * SUB: t * SUPER + (u + 1) * SUB, :],
                    ace[:, :])

        # software pipeline: prelude(t+1) is emitted before epilogue(t) so the
        # norm/inv chain of the next supertile overlaps this one's epilogue
        ybs = {}
        ahead = 2
        for t0 in range(min(ahead, n_super)):
            ybs[t0] = prelude(t0)
        for t in range(n_super):
            if t + ahead < n_super:
                ybs[t + ahead] = prelude(t + ahead)
            epilogue(t, ybs.pop(t))

        nc.vector.tensor_mul(ne_acc[:, :], ne_raw[:, :], inv_all[:, :])
        nc.sync.dma_start(se_d.ap()[:, :], se_acc[:, :])
        nc.sync.dma_start(ne_d.ap()[:, :], ne_acc[:, :])

    nc.compile()
    return nc


def _host_whiten(signatures, b_means, b_covs):
    try:
        import jax
        import jax.numpy as jnp
        with jax.default_device(jax.devices("cpu")[0]):
            cov = jnp.asarray(b_covs) @ jnp.asarray(b_covs).T
            U, eig, _ = jnp.linalg.svd(cov)
            DU = (eig ** -0.5)[:, None] * U.T
            W = DU.T
            mu_w = jnp.asarray(b_means) @ W
            s_w = jnp.asarray(signatures) @ W
            n = jnp.linalg.norm(s_w, axis=1, keepdims=True)
            sHat = s_w / jnp.maximum(n, 1e-12)
            return (np.ascontiguousarray(np.asarray(W, np.float32)),
                    np.asarray(mu_w, np.float32).reshape(1, D),
                    np.asarray(sHat, np.float32))
    except Exception:
        cov = b_covs @ b_covs.T
        U, eig, _ = np.linalg.svd(cov)
        DU = (eig ** -0.5)[:, None] * U.T
        W = np.ascontiguousarray(DU.T)
        mu_w = (b_means @ W).reshape(1, D)
        s_w = signatures @ W
        sHat = s_w / np.maximum(np.linalg.norm(s_w, axis=1, keepdims=True), 1e-12)
        return W, mu_w, sHat


def host_prep(X, labels, signatures, b_means, b_covs, n_rows=B_SHARD):
    """Returns in_maps for run_bass_kernel_spmd."""
    X = np.asarray(X, dtype=np.float32)
    labels = np.asarray(labels).astype(np.int32)
    signatures = np.asarray(signatures, dtype=np.float32)
    b_means = np.asarray(b_means, dtype=np.float32)
    b_covs = np.asarray(b_covs, dtype=np.float32)

    W, mu_w, sHat = _host_whiten(signatures, b_means, b_covs)
    SHT = np.ascontiguousarray(sHat.T)         # [D, C]

    wsb = np.ascontiguousarray(
        W.reshape(2, 128, 256).transpose(1, 0, 2).reshape(128, 512)
    ).astype(np.float16)
    sht = np.ascontiguousarray(
        SHT.reshape(2, 128, C).transpose(1, 0, 2).reshape(128, 2 * C)
    ).astype(np.float16)
    muw = np.ascontiguousarray((-mu_w).reshape(2, 128).T.astype(np.float32))
    iota = np.ascontiguousarray(
        np.tile(np.arange(C, dtype=np.float16)[None, :], (128, 1)))
    dmask = np.ascontiguousarray(np.tile(np.eye(128, dtype=np.float32), (1, 4)))

    n_sub = n_rows // SUB
    in_maps, perms = [], []
    sorted_lab = []
    for i in range(N_CORES):
        xs = X[i * n_rows:(i + 1) * n_rows]
        ls = labels[i * n_rows:(i + 1) * n_rows]
        perm = np.argsort(ls, kind="stable")
        perms.append(perm)
        xs = xs[perm]
        ls = ls[perm]
        sorted_lab.append(ls)
        xt = np.ascontiguousarray(xs.T.astype(np.float16)).reshape(2, 128, n_rows)
        lab = np.ascontiguousarray(ls.reshape(n_sub, 128).T.astype(np.float32))
        in_maps.append({"xt": xt, "wsb": wsb, "sht": sht, "muw": muw,
                        "lab": lab, "iota": iota, "dmask": dmask})
    sl = np.stack(sorted_lab)                       # [cores, n_rows]
    lo = sl.reshape(N_CORES, n_sub, SUB).min(axis=(0, 2))
    hi = sl.reshape(N_CORES, n_sub, SUB).max(axis=(0, 2)) + 1
    windows = tuple((int(a), int(b)) for a, b in zip(lo, hi))
    return in_maps, perms, windows


_CACHE = {}


def run_device(X, labels, signatures, b_means, b_covs, n_rows=B_SHARD,
               trace=False):
    in_maps, perms, windows = host_prep(X, labels, signatures, b_means,
                                        b_covs, n_rows)
    key = (n_rows, windows)
    if key not in _CACHE:
        _CACHE.clear()
        _CACHE[key] = build(n_rows, windows=windows)
    nc = _CACHE[key]
    res = run_bass_kernel_spmd(nc, in_maps, core_ids=list(range(N_CORES)),
                               trace=trace)
    return res, perms


def finish(res, perms, n_rows=B_SHARD):
    ace = np.empty((N_CORES * n_rows, C), np.float32)
    num = np.empty(N_CORES * n_rows, np.float64)
    se = np.empty(N_CORES * n_rows, np.float64)
    for i in range(N_CORES):
        perm = perms[i]
        sl = slice(i * n_rows, (i + 1) * n_rows)
        ace[sl.start:sl.stop][perm] = res.results[i]["ace"]
        num[sl][perm] = res.results[i]["numv"].flatten(order="F").astype(np.float64)
        se[sl][perm] = res.results[i]["sumexp"].flatten(order="F").astype(np.float64)
    loss = -np.mean(num - np.log(se))
    return np.float32(loss), ace


def kernel(X, labels, signatures, b_means, b_covs):
    res, perms = run_device(X, labels, signatures, b_means, b_covs)
    loss, ace = finish(res, perms)
    return loss, ace
